# revision 32
# baseline (speedup 1.0000x reference)
"""GCN classifier kernel for Trainium2 (Bass/Tile), 8-core SPMD. v3.

Math: per GCN layer, h' = relu(nd * (A^T (ns * h)) @ W + b)
  == relu(sum_e w_e * p[src_e] + b') per dst, with p = h @ W and
  w_e = ns[src_e] * nd[dst_e].

Key tricks:
- h1 = relu(q1*W0 + b0) is a function of the host-precomputable scalar
  q1 (layer-1 aggregate), so layer-2 messages need NO gather and NO
  table AllGather: msg = relu([q1[src]; 1]^T @ [W0; b0]) built on-device
  by K=2 outer-product matmuls (PE) + grouped ReLU (ScalarE). W1 is
  applied AFTER aggregation (one matmul per dst block).
- All matmul operands fp16 (1 PE cycle/row vs 4 for fp32).
- Layer-3 table AllGather split into 7 sub-collectives interleaved with
  the L2 block loop (table rows remapped so each lands contiguously).
- Gather groups are block-local with TIGHT table-row windows (edges
  sorted by row within a block, so each <=GA-chunk group spans a small
  row range; int16 indices are relative to the group's base row). Tile's
  range-based deps then let early gathers overlap L2: groups touching
  only completed sub-slabs issue between sub-collectives, bounded by
  the msg-tile pool (prefetch cap).
- Emission order batches each block's S tiles (one DVE
  tensor_scalar(is_equal, mult), fp16 fast mode) before its PE matmul
  chain; agg chains and readout matmuls are software-pipelined one
  block behind their producers to avoid cross-engine ping-pong stalls.
- Classifier head applied before the cross-core reduce: AllReduce moves
  [512,10] instead of [512,128].

Device pipeline per core (owns 6272 dst nodes = 49 blocks of 128):
  L2: per block: outer-product msgs + relu; aggT += msg^T@S (PE);
      h2T = relu(W1^T aggT + b1); p2 = h2 @ W2 -> slab2_k
      every 7 blocks: sub-AllGather slab2_k -> table2 slice,
      then prefetch gather groups covered by completed sub-slabs
  L3: dma_gather msgs = table2[base+idx]; agg += S^T@msg (+ ones x b2);
      h3 = relu(agg); r4 += h3^T @ Sg4 (one [128,512] readout matmul)
  head: out_t = (r4_t^T Wc) * invc_t; AllReduce [512,10]; + bc -> out
"""

import sys

sys.path.insert(0, "/opt/trn_rl_repo")

import numpy as np

import concourse.bass as bass
import concourse.mybir as mybir
import concourse.tile as tile
from concourse import bacc, bass_utils

P = 128
N_CORES = 8
N_NODES = 50000
N_EDGES = 800000
HID = 128
N_GRAPHS = 512
N_CLASSES = 10

NPC = 6272          # nodes per core (49 blocks of 128)
BLOCKS = NPC // P   # 49
NSUB = 7            # sub-slabs per core for chunked AllGather
SUBB = BLOCKS // NSUB   # 7 blocks per sub-slab
SUBR = SUBB * P         # 896 rows per sub-slab
SLABR = SUBR * N_CORES  # 7168 table rows per sub-slab
NPAD = NPC * N_CORES    # 50176
GA = 3              # gather group size in chunks of 128 edges
F32 = mybir.dt.float32
F16 = mybir.dt.float16
I16 = mybir.dt.int16
I32 = mybir.dt.int32


def _remap_rows(n):
    """Node id -> table2 row under the sub-slab-major AllGather layout."""
    c = n // NPC
    r = n % NPC
    k = r // SUBR
    return k * SLABR + c * SUBR + (r % SUBR)


def _prep_graph(src, dst, graph_ids, seq_idx=False):
    """Host-side preprocessing: degrees, q1, per-core edge schedule.

    Per core: edges bucketed by dst block, sorted by (remapped) src row.
    Chunks of 128 edges; gather groups of <=GA chunks, never crossing a
    block and capped to a <=32768-row window taken as the min/max over
    all cores (the SPMD program shares one AP per group). int16 indices
    are relative to the group's base row.
    """
    src = np.asarray(src).astype(np.int64)
    dst = np.asarray(dst).astype(np.int64)
    graph_ids = np.asarray(graph_ids).astype(np.int64)

    in_deg = np.bincount(dst, minlength=N_NODES).astype(np.float32)
    out_deg = np.bincount(src, minlength=N_NODES).astype(np.float32)
    ns = np.maximum(out_deg, 1.0) ** -0.5
    nd = np.maximum(in_deg, 1.0) ** -0.5
    c0 = (in_deg * ns).astype(np.float64)
    t1 = np.bincount(dst, weights=c0[src], minlength=N_NODES)
    q1 = (nd.astype(np.float64) * t1).astype(np.float32)

    w_edge = (ns[src] * nd[dst]).astype(np.float32)
    rsrc = _remap_rows(src)

    counts = np.zeros((N_CORES, BLOCKS), np.int64)
    per_core = []
    for c in range(N_CORES):
        base = c * NPC
        m = (dst >= base) & (dst < base + NPC)
        es, ed, ew, eq = rsrc[m], dst[m], w_edge[m], q1[src[m]]
        dloc = ed - base
        blk = dloc >> 7
        order = np.lexsort((es, blk))
        es, dloc, ew, eq, blk = (
            es[order], dloc[order], ew[order], eq[order], blk[order])
        for b in range(BLOCKS):
            counts[c, b] = np.count_nonzero(blk == b)
        per_core.append((es, dloc, ew, eq, blk))

    # chunks per block: max over cores (same shape on all cores, SPMD)
    K = np.maximum(1, np.ceil(counts / P).max(axis=0).astype(np.int64))
    KT = int(K.sum())
    off = np.concatenate([[0], np.cumsum(K)]).astype(int)

    core_arrays = []
    packed_rows = []
    for c in range(N_CORES):
        es, dloc, ew, eq, blk = per_core[c]
        rows = np.zeros(KT * P, np.int64)
        dv = np.zeros(KT * P, np.float32)
        w = np.zeros(KT * P, np.float32)
        q = np.zeros(KT * P, np.float32)
        for b in range(BLOCKS):
            m = blk == b
            n = int(np.count_nonzero(m))
            o = off[b] * P
            rows[o:o + n] = es[m]
            rows[o + n:off[b + 1] * P] = -1  # padding, fixed up below
            dv[o:o + n] = (dloc[m] - b * P).astype(np.float32)
            w[o:o + n] = ew[m]
            q[o:o + n] = eq[m]
        packed_rows.append(rows)
        core_arrays.append(dict(dv=dv, w=w, q=q))

    # padding rows: any in-window row works (w=0 kills the S term), but
    # they must not widen the cross-core group window. Use the per-chunk
    # max over all cores' REAL rows (cores with zero edges in a block
    # would otherwise pad with 0 and explode the window).
    allrows = np.stack(packed_rows)  # [8, KT*P]
    real = allrows >= 0
    fill = np.where(real, allrows, np.int64(-1)).reshape(N_CORES, KT, P)
    chunk_max = fill.max(axis=(0, 2))  # [KT] max real row per chunk
    # chunks with no real rows anywhere: walk back to previous value
    for ch in range(KT):
        if chunk_max[ch] < 0:
            chunk_max[ch] = chunk_max[ch - 1] if ch else 0
    pad_vals = np.repeat(chunk_max, P)
    for c in range(N_CORES):
        rows = packed_rows[c]
        rows[rows < 0] = pad_vals[rows < 0]

    # groups: <=GA chunks, block-local, cross-core row span <= 32768
    groups = []  # (block, chunk0, nchunks)
    for b in range(BLOCKS):
        g0 = off[b]
        while g0 < off[b + 1]:
            ln = 1
            while ln < GA and g0 + ln < off[b + 1]:
                lo = min(rows[g0 * P:(g0 + ln + 1) * P].min()
                         for rows in packed_rows)
                hi = max(rows[g0 * P:(g0 + ln + 1) * P].max()
                         for rows in packed_rows)
                if hi - lo >= 32768:
                    break
                ln += 1
            groups.append((b, g0, ln))
            g0 += ln
    NG = len(groups)
    gmaxlen = GA * P

    # per-group window = min/max over all cores (shared AP)
    gbase = np.zeros(NG, np.int64)
    gend = np.zeros(NG, np.int64)
    for gi, (b, g0, ln) in enumerate(groups):
        lo = min(rows[g0 * P:(g0 + ln) * P].min() for rows in packed_rows)
        hi = max(rows[g0 * P:(g0 + ln) * P].max() for rows in packed_rows)
        assert hi + 1 - lo <= 32768
        gbase[gi] = lo
        gend[gi] = hi + 1

    def idx_layout(v):
        r = v.astype(np.int16).reshape(-1, 16).T
        return np.tile(r, (8, 1)).copy()

    def col_layout(v):
        return np.ascontiguousarray(v.reshape(-1, P).T)

    for c in range(N_CORES):
        rows = packed_rows[c]
        idx = np.zeros((NG, gmaxlen), np.int64)
        for gi, (b, g0, ln) in enumerate(groups):
            seg = rows[g0 * P:(g0 + ln) * P]
            idx[gi, :ln * P] = seg - gbase[gi]
        ca = core_arrays[c]
        ca["idx"] = idx_layout(idx.reshape(-1))
        ca["dvc"] = col_layout(ca.pop("dv"))
        ca["wc"] = col_layout(ca.pop("w"))
        qv = ca.pop("q")
        ca["qone"] = np.ascontiguousarray(
            np.stack([qv, np.ones_like(qv)]).astype(np.float16))

        base = c * NPC
        own = np.arange(base, base + NPC)
        real = own < N_NODES
        gph = np.full(NPC, -1.0, np.float32)
        gph[real] = graph_ids[own[real]].astype(np.float32)
        ca["gphv"] = np.ascontiguousarray(gph.reshape(BLOCKS, P).T)

    cnt = np.bincount(graph_ids, minlength=N_GRAPHS).astype(np.float32)
    invc = (1.0 / np.maximum(cnt, 1.0)).reshape(N_GRAPHS // P, P).T
    invc = np.ascontiguousarray(invc)

    sched = dict(K=K, KT=KT, groups=groups, gbase=gbase, gend=gend)
    return sched, core_arrays, invc


def build_nc(sched, reps=1, with_coll=True, with_gather=True,
             with_sgen=True, with_compute=True, with_l2=True, with_l3=True,
             msg_bufs=130, sgen_bufs=40, hbuf_bufs=10, outer_bufs=3,
             msg2_bufs=10, s2_bufs=36, prefetch=120, dve_relu=0):
    """Build and compile the 8-core SPMD Bass program.

    prefetch: max gather groups issued interleaved with the L2
    sub-collective pipeline (bounded by msg pool depth).
    dve_relu=N: every Nth L2 msg-relu group runs on DVE (0 = all
    ScalarE).
    """
    K, KT = sched["K"], sched["KT"]
    groups, gbase, gend = sched["groups"], sched["gbase"], sched["gend"]
    NG = len(groups)
    NGT = N_GRAPHS // P
    off = np.concatenate([[0], np.cumsum(K)]).astype(int)
    MAXC = int(K.max())
    gmaxlen = GA * P

    nc = bacc.Bacc("TRN2", target_bir_lowering=False, debug=False,
                   num_devices=N_CORES, num_swdge_queues=4)

    def inp(name, shape, dt=F32):
        return nc.dram_tensor(name, list(shape), dt, kind="ExternalInput").ap()

    d_idx = inp("idx", [P, NG * gmaxlen // 16], I16)
    d_dv = inp("dvc", [P, KT])
    d_w = inp("wc", [P, KT])
    d_qone = inp("qone", [2, KT * P], F16)
    d_gph = inp("gphv", [P, BLOCKS])
    d_invc = inp("invc", [P, NGT])
    d_W0b0 = inp("W0b0", [2, HID], F16)
    d_W1 = inp("W1f", [HID, HID], F16)
    d_W2 = inp("W2f", [HID, HID], F16)
    d_Wc = inp("Wcf", [HID, N_CLASSES], F16)
    d_b1c = inp("b1c", [P, 1])
    d_b2r = inp("b2row", [1, HID], F16)
    d_bcr = inp("bcr", [P, N_CLASSES])

    out = nc.dram_tensor("out", [N_GRAPHS, N_CLASSES], F32,
                         kind="ExternalOutput").ap()

    slabs = [nc.dram_tensor(f"slab2_{k}", [SUBR, HID], F16,
                            kind="Internal").ap()
             for k in range(NSUB)]
    # double-buffered by rep parity: rep k+1's table writes must not
    # serialize behind rep k's gather reads (reps-pipelining)
    table2s = [nc.dram_tensor(f"table2_{i}", [NPAD, HID], F16,
                              kind="Internal", addr_space="Shared").ap()
               for i in range(2)]
    partials = [nc.dram_tensor(f"partial_{i}", [N_GRAPHS, N_CLASSES], F32,
                               kind="Internal").ap() for i in range(2)]
    summeds = [nc.dram_tensor(f"summed_{i}", [N_GRAPHS, N_CLASSES], F32,
                              kind="Internal", addr_space="Shared").ap()
               for i in range(2)]

    RG = [list(range(N_CORES))]

    with tile.TileContext(nc) as tc:
        with tc.tile_pool(name="const", bufs=1) as cp, \
             tc.tile_pool(name="msg", bufs=msg_bufs) as mp, \
             tc.tile_pool(name="sgen", bufs=sgen_bufs) as sp, \
             tc.tile_pool(name="msg2", bufs=msg2_bufs) as mp2, \
             tc.tile_pool(name="sg4", bufs=4) as sgp, \
             tc.tile_pool(name="s2", bufs=s2_bufs) as sp2, \
             tc.tile_pool(name="hbuf", bufs=hbuf_bufs) as hp, \
             tc.tile_pool(name="qblk", bufs=3) as qp, \
             tc.tile_pool(name="agg_ps", bufs=2, space="PSUM") as agg_ps, \
             tc.tile_pool(name="outer_ps", bufs=outer_bufs,
                          space="PSUM") as outer_ps, \
             tc.tile_pool(name="p_ps", bufs=2, space="PSUM") as p_ps, \
             tc.tile_pool(name="r_ps", bufs=1, space="PSUM") as r_ps:

            def load_const(ap_in, shape, dt=F32):
                t = cp.tile(list(shape), dt, tag=ap_in.name)
                nc.sync.dma_start(t[:], ap_in[:])
                return t

            idxt = load_const(d_idx, [P, NG * gmaxlen // 16], I16)
            dvt = load_const(d_dv, [P, KT])
            wt = load_const(d_w, [P, KT])
            gph = load_const(d_gph, [P, BLOCKS])
            invc = load_const(d_invc, [P, NGT])
            W0b0 = load_const(d_W0b0, [2, HID], F16)
            W1f = load_const(d_W1, [HID, HID], F16)
            W2f = load_const(d_W2, [HID, HID], F16)
            Wcf = load_const(d_Wc, [HID, N_CLASSES], F16)
            b1c = load_const(d_b1c, [P, 1])
            b2row = load_const(d_b2r, [1, HID], F16)
            bcr = load_const(d_bcr, [P, N_CLASSES])

            iota_i = cp.tile([P, P], I32, tag="iota_i")
            nc.gpsimd.iota(iota_i[:], pattern=[[1, P]], base=0,
                           channel_multiplier=0)
            iota_h = cp.tile([P, P], F16, tag="iota_h")
            nc.vector.tensor_copy(iota_h[:], iota_i[:])
            iotg_i = cp.tile([P, N_GRAPHS], I32, tag="iotg_i")
            nc.gpsimd.iota(iotg_i[:], pattern=[[1, N_GRAPHS]], base=0,
                           channel_multiplier=0)
            iotg_h = cp.tile([P, N_GRAPHS], F16, tag="iotg_h")
            nc.vector.tensor_copy(iotg_h[:], iotg_i[:])
            ones1 = cp.tile([1, P], F16, tag="ones1")
            nc.vector.memset(ones1[:], 1.0)

            RELU = mybir.ActivationFunctionType.Relu
            COPY = mybir.ActivationFunctionType.Copy
            EQ = mybir.AluOpType.is_equal
            MUL = mybir.AluOpType.mult
            MAX = mybir.AluOpType.max

            def sgen(pool, ci, tag="S"):
                S = pool.tile([P, P], F16, tag=tag)
                if with_sgen:
                    nc.vector.tensor_scalar(
                        out=S[:], in0=iota_h[:],
                        scalar1=dvt[:][:, ci:ci + 1],
                        scalar2=wt[:][:, ci:ci + 1],
                        op0=EQ, op1=MUL)
                return S

            ecount = [0]
            cur = {}

            def emit_gather(gi):
                b, g0, ln = groups[gi]
                mt = mp.tile([P, GA * P], F16, tag="msg")
                out_ap = mt[:][:, :ln * P].rearrange("p (a b) -> p a b", b=P)
                if with_gather:
                    ib = gi * (gmaxlen // 16)
                    nc.gpsimd.dma_gather(
                        out_ap=out_ap,
                        in_ap=cur["t2"][int(gbase[gi]):int(gend[gi]), :],
                        idxs_ap=idxt[:][:, ib:ib + ln * 8],
                        num_idxs=ln * P, num_idxs_reg=ln * P,
                        elem_size=HID, queue_num=ecount[0] % 4)
                    ecount[0] += 1
                else:
                    nc.vector.memset(mt[:], 0.25)
                return mt

            def l2_stage(b):
                nchunk = int(K[b])
                qblk = qp.tile([2, MAXC * P], F16, tag="qblk")
                nc.sync.dma_start(
                    qblk[:][:, :nchunk * P],
                    d_qone[:, off[b] * P:off[b + 1] * P])
                msgs = []
                Ss = []
                for g0 in range(0, nchunk, 4):
                    ln = min(4, nchunk - g0)
                    mp_ps = outer_ps.tile([P, 4 * P], F32, tag="outps")
                    for j in range(ln):
                        k = g0 + j
                        nc.tensor.matmul(
                            out=mp_ps[:][:, j * P:(j + 1) * P],
                            lhsT=qblk[:][:, k * P:(k + 1) * P],
                            rhs=W0b0[:], start=True, stop=True)
                    msg = mp2.tile([P, 4 * P], F16, tag="msg2")
                    if dve_relu and (g0 // 4) % dve_relu == dve_relu - 1:
                        nc.vector.tensor_scalar(
                            out=msg[:][:, :ln * P], in0=mp_ps[:][:, :ln * P],
                            scalar1=0.0, scalar2=None, op0=MAX)
                    else:
                        nc.scalar.activation(
                            out=msg[:][:, :ln * P], in_=mp_ps[:][:, :ln * P],
                            func=RELU)
                    msgs.append(msg)
                    for j in range(ln):
                        Ss.append(sgen(sp2, off[b] + g0 + j, tag="S2"))
                return b, msgs, Ss, nchunk

            def l2_finish(state):
                b, msgs, Ss, nchunk = state
                aggT = agg_ps.tile([P, P], F32, tag="aggps")
                for k in range(nchunk):
                    nc.tensor.matmul(
                        out=aggT[:],
                        lhsT=msgs[k // 4][:][:, (k % 4) * P:(k % 4 + 1) * P],
                        rhs=Ss[k][:],
                        start=(k == 0), stop=(k == nchunk - 1))
                aggT_sb = hp.tile([P, P], F16, tag="aggsb")
                nc.vector.tensor_copy(aggT_sb[:], aggT[:])
                h2_ps = p_ps.tile([P, P], F32, tag="pps")
                nc.tensor.matmul(out=h2_ps[:], lhsT=W1f[:],
                                 rhs=aggT_sb[:], start=True, stop=True)
                h2T = hp.tile([P, P], F16, tag="h2T")
                nc.scalar.activation(out=h2T[:], in_=h2_ps[:],
                                     func=RELU, bias=b1c[:])
                p2_ps = p_ps.tile([P, P], F32, tag="pps")
                nc.tensor.matmul(out=p2_ps[:], lhsT=h2T[:], rhs=W2f[:],
                                 start=True, stop=True)
                p2 = hp.tile([P, P], F16, tag="p2")
                nc.vector.tensor_copy(p2[:], p2_ps[:])
                k, rb = b // SUBB, b % SUBB
                nc.sync.dma_start(slabs[k][rb * P:(rb + 1) * P, :], p2[:])
                return k if rb == SUBB - 1 else None

            def emit_coll(k):
                if with_coll:
                    nc.gpsimd.collective_compute(
                        "AllGather", mybir.AluOpType.bypass,
                        replica_groups=RG, ins=[slabs[k][:]],
                        outs=[cur["t2"][k * SLABR:(k + 1) * SLABR, :]])

            # group -> last sub-slab its window reads
            gslab = [min(int(gend[gi] - 1) // SLABR, NSUB - 1)
                     for gi in range(NG)]
            # prefetch order: by finishing sub-slab, then index
            issue_order = sorted(range(NG), key=lambda gi: (gslab[gi], gi))

            for rep in range(reps):
                cur["t2"] = table2s[rep % 2]
                partial = partials[rep % 2]
                summed = summeds[rep % 2]
                chunk_mt = {}
                emitted = set()
                pos = [0]

                def emit_one(gi):
                    mt = emit_gather(gi)
                    _, g0, ln = groups[gi]
                    for j in range(ln):
                        chunk_mt[g0 + j] = (mt, j)
                    emitted.add(gi)

                def issue_upto(slab_done):
                    while (pos[0] < NG and len(emitted) < prefetch
                           and gslab[issue_order[pos[0]]] <= slab_done):
                        emit_one(issue_order[pos[0]])
                        pos[0] += 1

                # ---------------- layer 2 ----------------
                pend = None
                for b in range(BLOCKS if (with_compute and with_l2) else 0):
                    st = l2_stage(b)
                    if pend is not None:
                        kk = l2_finish(pend)
                        if kk is not None:
                            emit_coll(kk)
                            if with_l3:
                                issue_upto(kk)
                    pend = st
                if pend is not None:
                    kk = l2_finish(pend)
                    if kk is not None:
                        emit_coll(kk)

                # ---------------- layer 3 + readout ----------------
                if with_l3:
                    for gi in range(NG):
                        if gi not in emitted:
                            emit_one(gi)
                r4 = r_ps.tile([P, N_GRAPHS], F32, tag="rps",
                               name=f"rps_{rep}")
                pend3 = None
                for b in range(BLOCKS if (with_compute and with_l3) else 0):
                    nchunk = int(K[b])
                    Ss = [sgen(sp, off[b] + j, tag="S3")
                          for j in range(nchunk)]
                    Sg4 = sgp.tile([P, N_GRAPHS], F16, tag="Sg4")
                    nc.vector.tensor_scalar(
                        out=Sg4[:], in0=iotg_h[:],
                        scalar1=gph[:][:, b:b + 1], scalar2=None,
                        op0=EQ)
                    agg = agg_ps.tile([P, P], F32, tag="aggps")
                    for j in range(nchunk):
                        mt, col = chunk_mt[off[b] + j]
                        nc.tensor.matmul(
                            out=agg[:], lhsT=Ss[j][:],
                            rhs=mt[:][:, col * P:(col + 1) * P],
                            start=(j == 0), stop=False)
                    nc.tensor.matmul(out=agg[:], lhsT=ones1[:],
                                     rhs=b2row[:], start=False, stop=True)
                    h3 = hp.tile([P, P], F16, tag="h3")
                    nc.scalar.activation(out=h3[:], in_=agg[:], func=RELU)
                    if pend3 is not None:
                        h3p, Sg4p, bp = pend3
                        nc.tensor.matmul(out=r4[:], lhsT=h3p[:],
                                         rhs=Sg4p[:], start=(bp == 0),
                                         stop=False)
                    pend3 = (h3, Sg4, b)
                if pend3 is not None:
                    h3p, Sg4p, bp = pend3
                    nc.tensor.matmul(out=r4[:], lhsT=h3p[:], rhs=Sg4p[:],
                                     start=(bp == 0), stop=True)

                # ---------------- head + tiny AllReduce ----------------
                r4sb = hp.tile([P, N_GRAPHS], F16, tag="r4sb")
                if with_compute and with_l3:
                    nc.scalar.activation(out=r4sb[:], in_=r4[:], func=COPY)
                for t in range(NGT if (with_compute and with_l3) else 0):
                    o_ps = p_ps.tile([P, P], F32, tag="pps")
                    nc.tensor.matmul(
                        out=o_ps[:][:, :N_CLASSES],
                        lhsT=r4sb[:][:, t * P:(t + 1) * P],
                        rhs=Wcf[:], start=True, stop=True)
                    osb = hp.tile([P, N_CLASSES], F32, tag="osb")
                    nc.scalar.activation(out=osb[:],
                                         in_=o_ps[:][:, :N_CLASSES],
                                         func=COPY,
                                         scale=invc[:][:, t:t + 1])
                    nc.sync.dma_start(partial[t * P:(t + 1) * P, :], osb[:])

                if with_coll and with_compute and with_l3:
                    nc.gpsimd.collective_compute(
                        "AllReduce", mybir.AluOpType.add, replica_groups=RG,
                        ins=[partial[:]], outs=[summed[:]])

                for t in range(NGT if (with_compute and with_l3) else 0):
                    ld = hp.tile([P, N_CLASSES], F32, tag="ld")
                    nc.sync.dma_start(ld[:], summed[t * P:(t + 1) * P, :])
                    ob = hp.tile([P, N_CLASSES], F32, tag="ob")
                    nc.vector.tensor_tensor(out=ob[:], in0=ld[:], in1=bcr[:],
                                            op=mybir.AluOpType.add)
                    nc.sync.dma_start(out[t * P:(t + 1) * P, :], ob[:])

    nc.compile()
    return nc


def make_in_maps(core_arrays, invc, W0, b0, W1, b1, W2, b2, Wc, bc):
    W0b0 = np.stack([np.asarray(W0, np.float32).reshape(HID),
                     np.asarray(b0, np.float32).reshape(HID)]) \
        .astype(np.float16)
    common = dict(
        invc=np.ascontiguousarray(invc, np.float32),
        W0b0=np.ascontiguousarray(W0b0),
        W1f=np.ascontiguousarray(np.asarray(W1, np.float16)),
        W2f=np.ascontiguousarray(np.asarray(W2, np.float16)),
        Wcf=np.ascontiguousarray(np.asarray(Wc, np.float16)),
        b1c=np.ascontiguousarray(b1, np.float32).reshape(P, 1),
        b2row=np.ascontiguousarray(
            np.asarray(b2, np.float16).reshape(1, HID)),
        bcr=np.ascontiguousarray(np.tile(
            np.asarray(bc, np.float32).reshape(1, N_CLASSES), (P, 1))),
    )
    in_maps = []
    for c in range(N_CORES):
        m = dict(common)
        ca = core_arrays[c]
        for k in ("idx", "dvc", "wc", "qone", "gphv"):
            m[k] = ca[k]
        in_maps.append(m)
    return in_maps


_CACHE = {}


def _get_compiled(src, dst, graph_ids):
    import hashlib
    h = hashlib.md5()
    h.update(np.asarray(src).tobytes())
    h.update(np.asarray(dst).tobytes())
    h.update(np.asarray(graph_ids).tobytes())
    key = h.hexdigest()
    if key not in _CACHE:
        sched, core_arrays, invc = _prep_graph(src, dst, graph_ids)
        nc = build_nc(sched)
        _CACHE[key] = (nc, core_arrays, invc)
    return _CACHE[key]


def kernel(W0, b0, W1, b1, W2, b2, Wc, bc, src, dst, graph_ids,
           num_graphs=None, **_ignored):
    nc, core_arrays, invc = _get_compiled(src, dst, graph_ids)
    in_maps = make_in_maps(core_arrays, invc, W0, b0, W1, b1, W2, b2, Wc, bc)
    res = bass_utils.run_bass_kernel_spmd(
        nc, in_maps, core_ids=list(range(N_CORES)))
    return res.results[0]["out"]
